# revision 1
# baseline (speedup 1.0000x reference)
"""Trainium2 Bass kernel for nn_Loss_34230889349355 (superquadric fitting loss).

Sharding: data-parallel over batch B=8, one batch per NeuronCore.  Per core the
dominant work is the [P,S,N]=[16,200,4096] squared-distance tensor reduced by
min over S.  Using orthogonality of `rotate`, distances are computed in WORLD
frame so one set of PE weights (pc n-tile, K=4 with a ones row) serves all 16
primitives:

    D[n,(p,s)] = pc_n . (-2 X'_ps) + ||X'_ps||^2     (K=4 fp32r matmuls)
    min_s D + ||pc_n||^2 -> relu -> * assign -> sum

min over S: DVE pair-min fold tree (PSUM -> bf16 SBUF -> pair-min -> reduce),
with ScalarE prefetching half of each tile's PSUM into SBUF to split PSUM read
bandwidth.  Cuboid loss: per-primitive-frame coords from the same K=4 matmul
machinery (rhs = packed rotations), branchless first-wins face select.
Existence/sparsity losses only need assign column sums; those plus the two
weighted partial sums are shipped to the host ([128,18] per core) and the final
scalar combine happens on the host in float64.
"""

import numpy as np

B, N, P, S = 8, 4096, 16, 200
T = N // 128            # 32 n-tiles
HALF = P * S // 2       # 1600 D-columns per half-tile

_CACHE = {}


def _build():
    import concourse.bacc as bacc
    import concourse.tile as tile
    import concourse.bass as bass
    from concourse import mybir

    f32 = mybir.dt.float32
    f32r = mybir.dt.float32r
    bf16 = mybir.dt.bfloat16
    ALU = mybir.AluOpType
    ACT = mybir.ActivationFunctionType
    AX = mybir.AxisListType

    nc = bacc.Bacc(
        trn_type="TRN2",
        target_bir_lowering=False,
        debug=False,
        enable_asserts=False,
        num_devices=8,
    )

    pc_d = nc.dram_tensor("pc", [N, 3], f32, kind="ExternalInput")
    nr_d = nc.dram_tensor("normals", [N, 3], f32, kind="ExternalInput")
    tr_d = nc.dram_tensor("trans", [P, 3], f32, kind="ExternalInput")
    ro_d = nc.dram_tensor("rotate", [P, 3, 3], f32, kind="ExternalInput")
    sc_d = nc.dram_tensor("scale", [P, 3], f32, kind="ExternalInput")
    ep_d = nc.dram_tensor("shape_eps", [P, 2], f32, kind="ExternalInput")
    et_d = nc.dram_tensor("etas", [P, S], f32, kind="ExternalInput")
    om_d = nc.dram_tensor("omegas", [P, S], f32, kind="ExternalInput")
    as_d = nc.dram_tensor("assign_matrix", [N, P], f32, kind="ExternalInput")
    out_d = nc.dram_tensor("out", [128, 18], f32, kind="ExternalOutput")

    def dap(tns, ap, offset=0):
        return bass.AP(tensor=tns, offset=offset, ap=ap)

    with tile.TileContext(nc) as tc:
        with (
            tc.tile_pool(name="consts", bufs=1) as cp,
            tc.tile_pool(name="samp", bufs=1) as sp,
            tc.tile_pool(name="work", bufs=3) as wp,
            tc.tile_pool(name="cub", bufs=2) as bp,
            tc.tile_pool(name="psum", bufs=2, space="PSUM") as pp,
        ):
            # ------------- const APs for activation biases -------------
            czero = cp.tile([128, 2], f32)
            nc.vector.memset(czero[:, 0:1], 0.0)
            nc.vector.memset(czero[:, 1:2], float(np.pi / 2))
            nc.const_aps.aps[(f32, 0.0)] = czero[:, 0:1]
            nc.const_aps.aps[(f32, float(np.pi / 2))] = czero[:, 1:2]

            # ------------- input loads -------------
            pc4T = cp.tile([4, N], f32r)
            nc.vector.memset(pc4T.bitcast(f32), 1.0)  # row 3 stays all-ones
            for ch in range(4):
                nc.sync.dma_start(
                    out=pc4T[0:3, 1024 * ch: 1024 * (ch + 1)],
                    in_=dap(pc_d, [[1, 3], [3, 1024]], offset=3 * 1024 * ch).bitcast(f32r))

            nr4T = cp.tile([4, N], f32r)
            nc.vector.memset(nr4T.bitcast(f32), 1.0)
            for ch in range(4):
                nc.sync.dma_start(
                    out=nr4T[0:3, 1024 * ch: 1024 * (ch + 1)],
                    in_=dap(nr_d, [[1, 3], [3, 1024]], offset=3 * 1024 * ch).bitcast(f32r))

            # R4[j,(p,i)] = R_p[j,i] rows 0-2; row 3 = -(R_p^T t_p)_i
            R4 = cp.tile([4, P, 3], f32r)
            nc.sync.dma_start(out=R4[0:3, :, :], in_=dap(ro_d, [[3, 3], [9, P], [1, 3]]).bitcast(f32r))
            tbT = cp.tile([3, P], f32)      # tbT[j, p] = t_p[j]
            nc.sync.dma_start(out=tbT, in_=dap(tr_d, [[1, 3], [3, P]]))
            prod = cp.tile([3, P, 3], f32r)  # prod[j,(p,i)] = R_p[j,i] * t_p[j]
            for i in range(3):
                nc.vector.tensor_tensor(prod[:, :, i: i + 1], R4[0:3, :, i: i + 1],
                                        tbT.unsqueeze(2), ALU.mult)
            negones3 = cp.tile([3, 1], f32r)
            nc.vector.memset(negones3.bitcast(f32), -1.0)
            # row3 = -(sum_j prod[j]) via (-1)-matmul partition reduction, DMA'd
            # into partition 3 of R4 (compute engines can't start at partition 3).
            rpt = pp.tile([128, 2048], f32, tag="dps", name="rpt")
            nc.tensor.matmul(rpt[0:1, 0:48], negones3, prod,
                             start=True, stop=True)
            row3tmp = cp.tile([1, 48], f32r)
            nc.scalar.copy(row3tmp, rpt[0:1, 0:48])
            nc.sync.dma_start(out=R4[3:4, :, :],
                              in_=row3tmp.rearrange("a (p c) -> a p c", p=P))

            Rcols = cp.tile([P, 9], f32)
            nc.sync.dma_start(out=Rcols, in_=ro_d.ap().rearrange("p a b -> p (a b)"))
            tcols = cp.tile([P, 3], f32)
            nc.sync.dma_start(out=tcols, in_=tr_d.ap())
            acols = cp.tile([P, 3], f32)
            nc.sync.dma_start(out=acols, in_=sc_d.ap())
            ecols = cp.tile([P, 2], f32)
            nc.sync.dma_start(out=ecols, in_=ep_d.ap())
            etas = cp.tile([P, S], f32)
            nc.sync.dma_start(out=etas, in_=et_d.ap())
            omegas = cp.tile([P, S], f32)
            nc.sync.dma_start(out=omegas, in_=om_d.ap())

            scaleb3 = cp.tile([128, 8, P * 3], f32)  # scale bcast: partitions & tile-dim
            nc.sync.dma_start(out=scaleb3, in_=dap(sc_d, [[0, 128], [0, 8], [1, P * 3]]))

            A_all = cp.tile([128, T, P], f32)
            nc.sync.dma_start(out=A_all, in_=dap(as_d, [[P, 128], [128 * P, T], [1, P]]))

            pc_nt = cp.tile([128, T, 3], f32)
            nc.sync.dma_start(out=pc_nt, in_=dap(pc_d, [[3, 128], [128 * 3, T], [1, 3]]))

            pcI_all = cp.tile([128, T, P * 3], f32)
            nI_all = cp.tile([128, T, P * 3], f32)
            cdrel = cp.tile([128, T, P], f32)
            cubdiff = cp.tile([128, T, P], f32)
            rhs4 = cp.tile([4, P * S], f32r)
            minn_all = cp.tile([128, T, P], f32)
            out_sb = cp.tile([128, 18], f32)

            # ||pc_n||^2
            pc_sq = cp.tile([128, T, 3], f32)
            nc.scalar.activation(pc_sq, pc_nt, ACT.Square)
            pcn2 = cp.tile([128, T], f32)
            nc.vector.tensor_reduce(pcn2, pc_sq, axis=AX.X, op=ALU.add)

            # ------------- sampling -------------
            ce = sp.tile([P, S], f32)
            nc.scalar.activation(ce, etas, ACT.Sin, bias=float(np.pi / 2))
            se = sp.tile([P, S], f32)
            nc.scalar.activation(se, etas, ACT.Sin)
            co = sp.tile([P, S], f32)
            nc.scalar.activation(co, omegas, ACT.Sin, bias=float(np.pi / 2))
            so = sp.tile([P, S], f32)
            nc.scalar.activation(so, omegas, ACT.Sin)

            def fexp(v, e_col, nm):
                av = sp.tile([P, S], f32, tag=nm + "_a", name=nm + "_a")
                nc.scalar.activation(av, v, ACT.Abs)
                nc.scalar.activation(av, av, ACT.Ln)
                nc.vector.tensor_scalar(av, av, e_col, None, ALU.mult)
                ev = sp.tile([P, S], f32, tag=nm + "_e", name=nm + "_e")
                nc.scalar.activation(ev, av, ACT.Exp)
                sg = sp.tile([P, S], f32, tag=nm + "_s", name=nm + "_s")
                nc.scalar.activation(sg, v, ACT.Sign)
                nc.vector.tensor_tensor(ev, ev, sg, ALU.mult)
                return ev

            e1 = ecols[:, 0:1]
            e2 = ecols[:, 1:2]
            fce = fexp(ce, e1, "fce")
            fse = fexp(se, e1, "fse")
            fco = fexp(co, e2, "fco")
            fso = fexp(so, e2, "fso")

            def clampc(v, a_col, nm, mul_first=None):
                w = sp.tile([P, S], f32, tag=nm + "_w", name=nm + "_w")
                if mul_first is not None:
                    nc.vector.tensor_tensor(w, v, mul_first, ALU.mult)
                    nc.vector.tensor_scalar(w, w, a_col, None, ALU.mult)
                else:
                    nc.vector.tensor_scalar(w, v, a_col, None, ALU.mult)
                sg = sp.tile([P, S], f32, tag=nm + "_g", name=nm + "_g")
                nc.vector.tensor_scalar(sg, w, 0.0, None, ALU.is_gt)
                nc.vector.tensor_scalar(sg, sg, 2.0, -1.0, ALU.mult, ALU.add)
                aw = sp.tile([P, S], f32, tag=nm + "_m", name=nm + "_m")
                nc.scalar.activation(aw, w, ACT.Abs)
                nc.vector.tensor_scalar(aw, aw, 1e-6, None, ALU.max)
                nc.vector.tensor_tensor(w, sg, aw, ALU.mult)
                return w

            xc = clampc(fce, acols[:, 0:1], "xc", mul_first=fco)
            yc = clampc(fce, acols[:, 1:2], "yc", mul_first=fso)
            zc = clampc(fse, acols[:, 2:3], "zc")

            # X'' = -2(R X + t); rhs rows 0-2 = X''_i, row 3 = ||X''||^2/4
            R2 = sp.tile([P, 9], f32)
            nc.vector.tensor_scalar(R2, Rcols, -2.0, None, ALU.mult)
            t2 = sp.tile([P, 3], f32)
            nc.vector.tensor_scalar(t2, tcols, -2.0, None, ALU.mult)

            Xp = []
            for i in range(3):
                u = sp.tile([P, S], f32r, tag=f"xp{i}", name=f"xp{i}")
                nc.vector.tensor_scalar(u, xc, R2[:, 3 * i + 0: 3 * i + 1], None, ALU.mult)
                nc.vector.scalar_tensor_tensor(u, yc, R2[:, 3 * i + 1: 3 * i + 2], u, ALU.mult, ALU.add)
                nc.vector.scalar_tensor_tensor(u, zc, R2[:, 3 * i + 2: 3 * i + 3], u, ALU.mult, ALU.add)
                nc.vector.tensor_scalar(u, u, t2[:, i: i + 1], None, ALU.add)
                Xp.append(u)

            sq0 = sp.tile([P, S], f32r)
            nc.scalar.activation(sq0, Xp[0], ACT.Square)
            sq1 = sp.tile([P, S], f32r)
            nc.scalar.activation(sq1, Xp[1], ACT.Square)
            nc.vector.tensor_tensor(sq0, sq0, sq1, ALU.add)
            nc.scalar.activation(sq1, Xp[2], ACT.Square)
            nc.vector.tensor_tensor(sq0, sq0, sq1, ALU.add)
            nc.vector.tensor_scalar(sq0, sq0, 0.25, None, ALU.mult)

            for i, src_t in enumerate(Xp + [sq0]):
                nc.sync.dma_start(
                    out=rhs4[i: i + 1, :].rearrange("a (p s) -> a p s", p=P),
                    in_=src_t,
                )

            # ------------- transforms (pcI/nI) -------------
            R4f = R4.rearrange("a b c -> a (b c)")
            for g in range(8):
                pt = pp.tile([128, 2048], f32, tag="dps", name="pt")
                ptv = pt.rearrange("n (a x) -> n a x", a=4)
                for i in range(4):
                    t = 4 * g + i
                    nc.tensor.matmul(ptv[:, i, 0:48], pc4T[:, 128 * t: 128 * (t + 1)],
                                     R4f, start=True, stop=True)
                    nc.tensor.matmul(ptv[:, i, 64:112], nr4T[:, 128 * t: 128 * (t + 1)],
                                     R4f, start=True, stop=True)
                nc.scalar.copy(pcI_all[:, 4 * g: 4 * g + 4, :], ptv[:, :, 0:48])
                nc.vector.tensor_copy(nI_all[:, 4 * g: 4 * g + 4, :], ptv[:, :, 64:112])

            # ------------- cuboid group emitter (interleaved into main loop) -------------
            def emit_cuboid(g):
                sl = slice(8 * g, 8 * (g + 1))
                pcI = pcI_all[:, sl, :]
                nI = nI_all[:, sl, :]

                apc = bp.tile([128, 8, 48], f32, tag="apc", name="apc")
                nc.scalar.activation(apc, pcI, ACT.Abs)
                w1 = bp.tile([128, 8, 48], f32, tag="w1", name="w1")
                nc.vector.tensor_tensor(w1, apc, scaleb3, ALU.subtract)
                nc.scalar.activation(w1, w1, ACT.Relu)
                ee = bp.tile([128, 8, 48], f32, tag="ee", name="ee")
                nc.scalar.activation(ee, w1, ACT.Square)
                Ev = bp.tile([128, 8, P], f32, tag="Ev", name="Ev")
                nc.vector.tensor_reduce(Ev, ee.rearrange("n t (p c) -> n t p c", c=3),
                                        axis=AX.X, op=ALU.add)

                sgn = bp.tile([128, 8, 48], f32, tag="sgn", name="sgn")
                nc.scalar.activation(sgn, nI, ACT.Sign)
                qq = bp.tile([128, 8, 48], f32, tag="qq", name="qq")
                nc.vector.tensor_tensor(qq, sgn, pcI, ALU.mult)
                nc.vector.tensor_tensor(qq, qq, scaleb3, ALU.subtract)
                nc.scalar.activation(qq, qq, ACT.Square)
                dd = bp.tile([128, 8, 48], f32, tag="dd", name="dd")
                nc.vector.tensor_tensor(dd, qq, ee, ALU.subtract)

                tA = bp.tile([128, 8, 48], f32, tag="tA", name="tA")
                nc.scalar.activation(tA, nI, ACT.Abs)
                tAv = tA.rearrange("n t (p c) -> n t p c", c=3)
                ddv = dd.rearrange("n t (p c) -> n t p c", c=3)
                c1 = bp.tile([128, 8, P], mybir.dt.uint8, tag="c1", name="c1")
                nc.vector.tensor_tensor(c1, tAv[:, :, :, 0], tAv[:, :, :, 1], ALU.is_ge)
                t1 = bp.tile([128, 8, P], f32, tag="t1", name="t1")
                nc.vector.tensor_tensor(t1, tAv[:, :, :, 0], tAv[:, :, :, 1], ALU.max)
                d1 = bp.tile([128, 8, P], f32, tag="d1", name="d1")
                nc.vector.select(d1, c1, ddv[:, :, :, 0], ddv[:, :, :, 1])
                c2 = bp.tile([128, 8, P], mybir.dt.uint8, tag="c2", name="c2")
                nc.vector.tensor_tensor(c2, t1, tAv[:, :, :, 2], ALU.is_ge)
                dsel = bp.tile([128, 8, P], f32, tag="dsel", name="dsel")
                nc.vector.select(dsel, c2, d1, ddv[:, :, :, 2])
                nc.vector.tensor_tensor(cubdiff[:, sl, :], Ev, dsel, ALU.add)

            # ------------- main loop: D + min-fold -------------
            for t in range(T):
                lhs_pc = pc4T[:, 128 * t: 128 * (t + 1)]
                f1 = wp.tile([128, P, 100], bf16, tag="f1", name="f1")
                for h in range(2):
                    dt = pp.tile([128, 2048], f32, tag="dps", name="dt")
                    dv = dt.rearrange("n (a x) -> n a x", a=4)
                    for q in range(4):
                        nc.tensor.matmul(
                            dv[:, q, 0:400], lhs_pc,
                            rhs4[:, HALF * h + 400 * q: HALF * h + 400 * (q + 1)],
                            start=True, stop=True,
                        )
                    dpair = dv[:, :, 0:400].rearrange("n a (b s) -> n a b s", b=2)
                    f1v = f1[:, 8 * h: 8 * h + 8, :].rearrange("n (a b) s -> n a b s", a=4)
                    if h == 1:
                        # ScalarE copies the whole half to SBUF; DVE folds at 2x
                        sbD = wp.tile([128, 4, 400], bf16, tag="sbD", name="sbD")
                        nc.scalar.copy(sbD, dv[:, :, 0:400])
                        sv = sbD.rearrange("n a (b s) -> n a b s", b=2)
                        nc.vector.tensor_tensor(f1v, sv[:, :, :, 0:100], sv[:, :, :, 100:200], ALU.min)
                    else:
                        # TT may read only one input from PSUM: ScalarE copies the
                        # odd sub-halves to SBUF, DVE folds PSUM vs SBUF.
                        sbO = wp.tile([128, 4, 2, 100], bf16, tag="sbO", name="sbO")
                        nc.scalar.copy(sbO, dpair[:, :, :, 100:200])
                        nc.vector.tensor_tensor(f1v, dpair[:, :, :, 0:100], sbO, ALU.min)
                f2 = wp.tile([128, P, 50], bf16, tag="f2", name="f2")
                nc.vector.tensor_tensor(f2, f1[:, :, 0:50], f1[:, :, 50:100], ALU.min)
                nc.vector.tensor_reduce(minn_all[:, t, :], f2, axis=AX.X, op=ALU.min)


            # relu(min + ||pc||^2) for all tiles, off the ACT critical path
            for t in range(T):
                nc.scalar.activation(cdrel[:, t, :], minn_all[:, t, :], ACT.Relu,
                                     bias=pcn2[:, t: t + 1])

            for g in range(4):
                emit_cuboid(g)

            # ------------- final partial sums -------------
            scr = cp.tile([128, T * P], f32)
            nc.vector.scalar_tensor_tensor(
                scr, cdrel.rearrange("n t p -> n (t p)"), 1.0,
                A_all.rearrange("n t p -> n (t p)"), ALU.mult, ALU.mult,
                accum_out=out_sb[:, 0:1])
            nc.vector.scalar_tensor_tensor(
                scr, cubdiff.rearrange("n t p -> n (t p)"), 1.0,
                A_all.rearrange("n t p -> n (t p)"), ALU.mult, ALU.mult,
                accum_out=out_sb[:, 1:2])
            nc.vector.tensor_reduce(out_sb[:, 2:18], A_all.rearrange("n t p -> n p t"),
                                    axis=AX.X, op=ALU.add)
            nc.sync.dma_start(out=out_d.ap(), in_=out_sb)

    nc.compile()
    return nc


def _get_nc():
    if "nc" not in _CACHE:
        _CACHE["nc"] = _build()
    return _CACHE["nc"]


def kernel(**inputs):
    import concourse.bass_utils as bass_utils

    nc = _get_nc()
    names = ["pc", "normals", "trans", "rotate", "scale", "shape_eps",
             "etas", "omegas", "assign_matrix"]
    in_maps = []
    for b in range(B):
        in_maps.append({
            k: np.ascontiguousarray(np.asarray(inputs[k][b], dtype=np.float32))
            for k in names
        })
    res = bass_utils.run_bass_kernel_spmd(nc, in_maps, core_ids=list(range(8)))

    cd_sums, cub_sums, colsums = [], [], []
    for b in range(B):
        o = np.asarray(res.results[b]["out"], dtype=np.float64)
        cd_sums.append(o[:, 0].sum())
        cub_sums.append(o[:, 1].sum())
        colsums.append(o[:, 2:18].sum(axis=0))

    cub = np.sum(cub_sums) / (B * N)
    cd = 2.0 * np.sum(cd_sums) / (B * N)
    ext_terms, sps_terms = [], []
    exist = np.asarray(inputs["exist"], dtype=np.float64)
    for b in range(B):
        gt = (colsums[b] > 24.0).astype(np.float64)
        pr = exist[b, :, 0]
        bce = -(gt * np.maximum(np.log(pr), -100.0)
                + (1 - gt) * np.maximum(np.log(1.0 - pr), -100.0))
        ext_terms.append(bce.mean())
        sps_terms.append(np.sqrt(colsums[b] / N + 0.01).mean() ** 2)
    ext = float(np.mean(ext_terms))
    sps = float(np.mean(sps_terms))
    loss = 1.0 * cub + 1.0 * cd + 0.1 * ext + 0.1 * sps
    return np.float32(loss)



# revision 7
# speedup vs baseline: 1.2501x; 1.2501x over previous
"""Trainium2 Bass kernel for nn_Loss_34230889349355 (superquadric fitting loss).

Sharding: data-parallel over batch B=8, one batch per NeuronCore.  Per core the
dominant work is the [P,S,N]=[16,200,4096] squared-distance tensor reduced by
min over S.  Distances are computed in WORLD frame (rotate is orthonormal) via
K=5 fp32r matmuls whose lhs rows are [x,y,z,1,||x||^2] and rhs rows are
[-2X', ||X'||^2, 1], so PSUM holds the full squared distance (>= 0) and no
post-min bias/relu pass is needed.

Small per-primitive tensors (superquadric surface samples, rotations, fused
rhs) are precomputed on the host - they are O(P*S) and feed the device as
plain DMA inputs, removing the trig/pow sampling stage and its activation
table loads entirely.

min over S per tile (16p x 200s = 3200 PSUM f32), engine-balanced under the
hardware rules (PSUM readable only by ACT, and by DVE with one PSUM operand
per instruction; GPSIMD cannot execute generic tensor ops in this flow):
  ACT copies h0 (8p, full depth) and the s-hi half of h1 to fp16 SBUF;
  DVE pair-mins h1's s-lo (PSUM) against the copies, then runs the packed
  fp16 2x fold chain 200->100->50->25 and a final 25-deep min-reduce.
Cuboid loss: primitive-frame coords from the same K=5 matmuls (rhs = packed
rotations) in PLANAR (xxx..yyy..zzz) layout so every select-layer op is
packed fp16 (2x DVE); squares/abs/sign/relu on ACT, axis-sum as two adds.
Existence/sparsity only need assign column sums; those plus the two weighted
partial sums ship to the host ([128,18] per core); final scalar combine in
float64 on host.
"""

import numpy as np

B, N, P, S = 8, 4096, 16, 200
T = N // 128            # 32 n-tiles
PS = P * S              # 3200 D-columns

_CACHE = {}


def _build():
    import concourse.bacc as bacc
    import concourse.tile as tile
    import concourse.bass as bass
    from concourse import mybir

    f32 = mybir.dt.float32
    f32r = mybir.dt.float32r
    f16 = mybir.dt.float16
    ALU = mybir.AluOpType
    ACT = mybir.ActivationFunctionType
    AX = mybir.AxisListType

    nc = bacc.Bacc(
        trn_type="TRN2",
        target_bir_lowering=False,
        debug=False,
        enable_asserts=False,
        num_devices=8,
    )

    pc5_d = nc.dram_tensor("pc5", [5, N], f32, kind="ExternalInput")
    nr5_d = nc.dram_tensor("nr5", [5, N], f32, kind="ExternalInput")
    r5_d = nc.dram_tensor("r5", [5, P * 3], f32, kind="ExternalInput")
    rhs5_d = nc.dram_tensor("rhs5", [5, PS], f32, kind="ExternalInput")
    scl_d = nc.dram_tensor("scl16", [P * 3], f16, kind="ExternalInput")
    as_d = nc.dram_tensor("assign_matrix", [N, P], f32, kind="ExternalInput")
    out_d = nc.dram_tensor("out", [128, 18], f32, kind="ExternalOutput")

    def dap(tns, ap, offset=0):
        return bass.AP(tensor=tns, offset=offset, ap=ap)

    with tile.TileContext(nc) as tc:
        with (
            tc.tile_pool(name="consts", bufs=1) as cp,
            tc.tile_pool(name="cub", bufs=1) as bp,
            tc.tile_pool(name="wc", bufs=3) as wcp,
            tc.tile_pool(name="ww", bufs=3) as wwp,
            tc.tile_pool(name="wh", bufs=3) as whp,
            tc.tile_pool(name="wg", bufs=3) as wgp,
            tc.tile_pool(name="psum", bufs=2, space="PSUM") as pp,
        ):
            # const AP for activation bias 0.0
            czero = cp.tile([128, 1], f32)
            nc.vector.memset(czero, 0.0)
            nc.const_aps.aps[(f32, 0.0)] = czero

            # ------------- input loads -------------
            pc5 = cp.tile([5, N], f32r)
            nc.sync.dma_start(out=pc5, in_=pc5_d.ap().bitcast(f32r))
            rhs5 = cp.tile([5, PS], f32r)
            nc.sync.dma_start(out=rhs5, in_=rhs5_d.ap().bitcast(f32r))
            R5f = cp.tile([5, P * 3], f32r)
            nc.sync.dma_start(out=R5f, in_=r5_d.ap().bitcast(f32r))
            nr5 = cp.tile([5, N], f32r)
            nc.sync.dma_start(out=nr5, in_=nr5_d.ap().bitcast(f32r))
            scaleb3 = cp.tile([128, T, P * 3], f16)
            nc.sync.dma_start(out=scaleb3, in_=dap(scl_d, [[0, 128], [0, T], [1, P * 3]]))
            A_all = cp.tile([128, T, P], f32)
            nc.sync.dma_start(out=A_all, in_=dap(as_d, [[P, 128], [128 * P, T], [1, P]]))

            pcI_all = cp.tile([128, T, P * 3], f16)
            nI_all = cp.tile([128, T, P * 3], f16)
            minn_all = cp.tile([128, T, P], f16)
            cubdiff = cp.tile([128, T, P], f16)
            out_sb = cp.tile([128, 18], f32)

            # ------------- transforms (pcI/nI, primitive frame, planar) ----
            for g in range(8):
                pt = pp.tile([128, 2048], f32, tag="dps", name="pt")
                ptv = pt.rearrange("n (a x) -> n a x", a=4)
                for i in range(4):
                    t = 4 * g + i
                    nc.tensor.matmul(ptv[:, i, 0:48], pc5[:, 128 * t: 128 * (t + 1)],
                                     R5f, start=True, stop=True)
                    nc.tensor.matmul(ptv[:, i, 64:112], nr5[:, 128 * t: 128 * (t + 1)],
                                     R5f, start=True, stop=True)
                nc.scalar.copy(pcI_all[:, 4 * g: 4 * g + 4, :], ptv[:, :, 0:48])
                nc.scalar.copy(nI_all[:, 4 * g: 4 * g + 4, :], ptv[:, :, 64:112])

            # ------------- main loop -------------
            for t in range(T):
                lhs_pc = pc5[:, 128 * t: 128 * (t + 1)]
                d0 = pp.tile([128, 2048], f32, tag="dps", name="d0")
                d0v = d0.rearrange("n (a x) -> n a x", a=4)
                for q in range(4):
                    nc.tensor.matmul(d0v[:, q, 0:400], lhs_pc,
                                     rhs5[:, 400 * q: 400 * (q + 1)],
                                     start=True, stop=True)
                d1 = pp.tile([128, 2048], f32, tag="dps", name="d1")
                d1v = d1.rearrange("n (a x) -> n a x", a=4)
                for q in range(4):
                    nc.tensor.matmul(d1v[:, q, 0:400], lhs_pc,
                                     rhs5[:, 1600 + 400 * q: 1600 + 400 * (q + 1)],
                                     start=True, stop=True)

                # ACT: evacuate h0 (8p full depth) to fp16
                C = wcp.tile([128, 8, 200], f16, tag="C", name="C")
                nc.scalar.copy(C.rearrange("n (a p) s -> n a (p s)", a=4),
                               d0v[:, :, 0:400])
                # ACT: evacuate h1's s-hi halves
                Chi = wcp.tile([128, 4, 2, 100], f16, tag="Chi", name="Chi")
                d1p = d1v[:, :, 0:400].rearrange("n q (p s) -> n q p s", p=2)
                nc.scalar.copy(Chi, d1p[:, :, :, 100:200])

                W = wwp.tile([128, 16, 100], f16, tag="W", name="W")
                # DVE: pair-min h1 s-lo (PSUM) vs the s-hi copies
                nc.vector.tensor_tensor(
                    W[:, 8:16, :].rearrange("n (q p) s -> n q p s", q=4),
                    d1p[:, :, :, 0:100], Chi, ALU.min)
                # DVE: fold C -> depth 100 (packed fp16, 2x)
                nc.vector.tensor_tensor(W[:, 0:8, :], C[:, :, 0:100],
                                        C[:, :, 100:200], ALU.min)
                H = whp.tile([128, 16, 50], f16, tag="H", name="H")
                nc.vector.tensor_tensor(H, W[:, :, 0:50], W[:, :, 50:100], ALU.min)
                G = wgp.tile([128, 16, 25], f16, tag="G", name="G")
                nc.vector.tensor_tensor(G, H[:, :, 0:25], H[:, :, 25:50], ALU.min)
                nc.vector.tensor_reduce(minn_all[:, t, :], G, axis=AX.X, op=ALU.min)

            # ------------- cuboid loss (planar fp16) -------------
            apc = bp.tile([128, T, 48], f16)
            nc.scalar.activation(apc, pcI_all, ACT.Abs)
            sgn = bp.tile([128, T, 48], f16)
            nc.scalar.activation(sgn, nI_all, ACT.Sign)
            tA = bp.tile([128, T, 48], f16)
            nc.scalar.activation(tA, nI_all, ACT.Abs)
            u = bp.tile([128, T, 48], f16)
            nc.vector.tensor_tensor(u, apc, scaleb3, ALU.subtract)
            r = bp.tile([128, T, 48], f16)
            nc.scalar.activation(r, u, ACT.Relu)
            v = bp.tile([128, T, 48], f16)
            nc.scalar.activation(v, r, ACT.Square)
            m1 = bp.tile([128, T, 48], f16)
            nc.vector.tensor_tensor(m1, sgn, pcI_all, ALU.mult)
            q2 = bp.tile([128, T, 48], f16)
            nc.vector.tensor_tensor(q2, m1, scaleb3, ALU.subtract)
            w2 = bp.tile([128, T, 48], f16)
            nc.scalar.activation(w2, q2, ACT.Square)
            dd = bp.tile([128, T, 48], f16)
            nc.vector.tensor_tensor(dd, w2, v, ALU.subtract)

            # planar views: [:, :, 16*i : 16*(i+1)] is axis i, packed
            Ev = bp.tile([128, T, P], f16)
            with nc.allow_low_precision(reason="3-term fp16 sum, |v| < ~100"):
                nc.vector.tensor_tensor(Ev, v[:, :, 0:16], v[:, :, 16:32], ALU.add)
                nc.vector.tensor_tensor(Ev, Ev, v[:, :, 32:48], ALU.add)
            c1 = bp.tile([128, T, P], mybir.dt.uint8)
            nc.vector.tensor_tensor(c1, tA[:, :, 0:16], tA[:, :, 16:32], ALU.is_ge)
            t1 = bp.tile([128, T, P], f16)
            nc.vector.tensor_tensor(t1, tA[:, :, 0:16], tA[:, :, 16:32], ALU.max)
            d1s = bp.tile([128, T, P], f16)
            nc.vector.select(d1s, c1, dd[:, :, 0:16], dd[:, :, 16:32])
            c2 = bp.tile([128, T, P], mybir.dt.uint8)
            nc.vector.tensor_tensor(c2, t1, tA[:, :, 32:48], ALU.is_ge)
            dsel = bp.tile([128, T, P], f16)
            nc.vector.select(dsel, c2, d1s, dd[:, :, 32:48])
            with nc.allow_low_precision(reason="fp16 cuboid partials"):
                nc.vector.tensor_tensor(cubdiff, Ev, dsel, ALU.add)

            # ------------- final partial sums -------------
            # numeric insurance: clamp tiny negative mins (fp32r noise) at 0
            nc.vector.tensor_scalar(minn_all.rearrange("n t p -> n (t p)"),
                                    minn_all.rearrange("n t p -> n (t p)"),
                                    0.0, None, ALU.max)
            scr = cp.tile([128, T * P], f16)
            nc.vector.scalar_tensor_tensor(
                scr, minn_all.rearrange("n t p -> n (t p)"), 1.0,
                A_all.rearrange("n t p -> n (t p)"), ALU.mult, ALU.mult,
                accum_out=out_sb[:, 0:1])
            nc.vector.scalar_tensor_tensor(
                scr, cubdiff.rearrange("n t p -> n (t p)"), 1.0,
                A_all.rearrange("n t p -> n (t p)"), ALU.mult, ALU.mult,
                accum_out=out_sb[:, 1:2])
            nc.vector.tensor_reduce(out_sb[:, 2:18], A_all.rearrange("n t p -> n p t"),
                                    axis=AX.X, op=ALU.add)
            nc.sync.dma_start(out=out_d.ap(), in_=out_sb)

    nc.compile()
    return nc


def _get_nc():
    if "nc" not in _CACHE:
        _CACHE["nc"] = _build()
    return _CACHE["nc"]


def _host_prep(inputs):
    """Per-batch input marshalling: superquadric surface samples, fused
    matmul operands.  All O(P*S) work."""
    f32 = np.float32
    in_maps = []
    for b in range(B):
        pc = np.asarray(inputs["pc"][b], dtype=np.float64)
        nr = np.asarray(inputs["normals"][b], dtype=np.float64)
        R = np.asarray(inputs["rotate"][b], dtype=np.float64)
        tr = np.asarray(inputs["trans"][b], dtype=np.float64)
        sc = np.asarray(inputs["scale"][b], dtype=np.float64)
        ep = np.asarray(inputs["shape_eps"][b], dtype=np.float64)
        et = np.asarray(inputs["etas"][b], dtype=np.float64)
        om = np.asarray(inputs["omegas"][b], dtype=np.float64)

        et = np.where(et == 0, 1e-6, et)
        om = np.where(om == 0, 1e-6, om)
        fexp = lambda x, p: np.sign(x) * np.abs(x) ** p
        ce, se = np.cos(et), np.sin(et)
        co, so = np.cos(om), np.sin(om)
        e1, e2 = ep[:, 0:1], ep[:, 1:2]
        x = sc[:, 0:1] * fexp(ce, e1) * fexp(co, e2)
        y = sc[:, 1:2] * fexp(ce, e1) * fexp(so, e2)
        z = sc[:, 2:3] * fexp(se, e1)
        clamp = lambda v: ((v > 0) * 2.0 - 1.0) * np.maximum(np.abs(v), 1e-6)
        X = np.stack([clamp(x), clamp(y), clamp(z)], -1)        # [P,S,3]
        Xw = np.einsum("pij,psj->psi", R, X) + tr[:, None, :]   # [P,S,3] world

        rhs5 = np.empty((5, PS), f32)
        rhs5[0:3] = (-2.0 * Xw).reshape(PS, 3).T
        rhs5[3] = (Xw ** 2).sum(-1).reshape(PS)
        rhs5[4] = 1.0

        pc5 = np.empty((5, N), f32)
        pc5[0:3] = pc.T
        pc5[3] = 1.0
        pc5[4] = (pc ** 2).sum(-1)

        nr5 = np.empty((5, N), f32)
        nr5[0:3] = nr.T
        nr5[3] = 1.0
        nr5[4] = 0.0

        # planar rotation rhs: col = 16*i + p  ->  out pcI[:, :, 16i+p] = axis i
        r5 = np.empty((5, 3, P), f32)
        r5[0:3] = np.transpose(R, (1, 2, 0))                    # r5[j,i,p]=R[p,j,i]
        r5[3] = -np.einsum("pji,pj->ip", R, tr)                 # -(R^T t), planar
        r5[4] = 0.0

        in_maps.append({
            "pc5": pc5,
            "nr5": nr5,
            "r5": np.ascontiguousarray(r5.reshape(5, P * 3)),
            "rhs5": rhs5,
            "scl16": np.ascontiguousarray(sc.T.reshape(P * 3)).astype(np.float16),
            "assign_matrix": np.ascontiguousarray(np.asarray(inputs["assign_matrix"][b], dtype=f32)),
        })
    return in_maps


def kernel(**inputs):
    import concourse.bass_utils as bass_utils

    nc = _get_nc()
    in_maps = _host_prep(inputs)
    res = bass_utils.run_bass_kernel_spmd(nc, in_maps, core_ids=list(range(8)))

    cd_sums, cub_sums, colsums = [], [], []
    for b in range(B):
        o = np.asarray(res.results[b]["out"], dtype=np.float64)
        cd_sums.append(o[:, 0].sum())
        cub_sums.append(o[:, 1].sum())
        colsums.append(o[:, 2:18].sum(axis=0))

    cub = np.sum(cub_sums) / (B * N)
    cd = 2.0 * np.sum(cd_sums) / (B * N)
    ext_terms, sps_terms = [], []
    exist = np.asarray(inputs["exist"], dtype=np.float64)
    for b in range(B):
        gt = (colsums[b] > 24.0).astype(np.float64)
        pr = exist[b, :, 0]
        bce = -(gt * np.maximum(np.log(pr), -100.0)
                + (1 - gt) * np.maximum(np.log(1.0 - pr), -100.0))
        ext_terms.append(bce.mean())
        sps_terms.append(np.sqrt(colsums[b] / N + 0.01).mean() ** 2)
    ext = float(np.mean(ext_terms))
    sps = float(np.mean(sps_terms))
    loss = 1.0 * cub + 1.0 * cd + 0.1 * ext + 0.1 * sps
    return np.float32(loss)
